# revision 3
# baseline (speedup 1.0000x reference)
"""MoE feed-forward (8 experts, top-2, SwiGLU) on 8 Trainium2 NeuronCores.

Strategy: expert parallelism. Core c owns expert c and computes its expert's
SwiGLU output for all tokens with fp32r (FP22) matmuls, weights resident in
SBUF. Gating (router top-2 softmax) is computed on host in float64 and the
per-expert gating row is shipped as an input; each core scales its expert
output by its gating row, partial outputs are combined with an on-device
ReduceScatter, and the host reassembles the full output.
"""

import os
import sys
import time

sys.path.insert(0, "/opt/trn_rl_repo")

import numpy as np

# ---------------------------------------------------------------------------
# Problem constants (hardcoded per contract)
B, S, D, E, I, TOPK = 2, 2048, 1024, 8, 1408, 2
T = B * S  # 4096 tokens
P = 128
D_T = D // P   # 8 d-tiles
I_T = I // P   # 11 i-tiles
TC = 256       # token chunk (PSUM-bank free dim)
N_CORES = 8

_VERBOSE = bool(int(os.environ.get("KERNEL_VERBOSE", "0")))


def _log(msg):
    if _VERBOSE:
        print(f"[kernel] {msg}", flush=True)


def round_f32r(a: np.ndarray) -> np.ndarray:
    """RNE-round fp32 array to 13 mantissa bits (FP22 / e8m13)."""
    v = np.ascontiguousarray(a, dtype=np.float32).view(np.uint32)
    low = v & np.uint32(0x1FFF)
    base = v & np.uint32(0xFFFFE000)
    lsb = (v >> np.uint32(13)) & np.uint32(1)
    round_up = (low > np.uint32(0x1000)) | ((low == np.uint32(0x1000)) & (lsb == 1))
    out = base + (round_up.astype(np.uint32) << np.uint32(13))
    return out.view(np.float32)


def host_gating(x2d: np.ndarray, gate_w: np.ndarray):
    """Exact router: scores -> top-2 -> softmax. Returns gating [T, E] fp32."""
    scores = x2d.astype(np.float64) @ gate_w.astype(np.float64).T  # [T, E]
    idx = np.argsort(-scores, axis=-1, kind="stable")[:, :TOPK]  # [T, 2]
    top = np.take_along_axis(scores, idx, axis=-1)  # [T, 2] descending
    m = top[:, :1]
    ex = np.exp(top - m)
    probs = ex / ex.sum(axis=-1, keepdims=True)  # [T, 2]
    gating = np.zeros((x2d.shape[0], E), dtype=np.float64)
    np.put_along_axis(gating, idx, probs, axis=-1)
    return gating.astype(np.float32)


# ---------------------------------------------------------------------------
# Bass kernel builder


def build_nc(t_total=T, tc=TC, n_cores=N_CORES):
    import concourse.bass as bass  # noqa: F401
    import concourse.mybir as mybir
    import concourse.tile as tile
    from concourse import bacc

    f32 = mybir.dt.float32
    f32r = mybir.dt.float32r
    n_chunks = t_total // tc

    nc = bacc.Bacc("TRN2", debug=False, num_devices=n_cores)

    xT_d = nc.dram_tensor("xT", [D, t_total], f32r, kind="ExternalInput")
    wgT_d = nc.dram_tensor("wgT", [D, I], f32r, kind="ExternalInput")
    wuT_d = nc.dram_tensor("wuT", [D, I], f32r, kind="ExternalInput")
    wdT_d = nc.dram_tensor("wdT", [I, D], f32r, kind="ExternalInput")
    gcol_d = nc.dram_tensor("gcol", [1, t_total], f32r, kind="ExternalInput")
    ones_d = nc.dram_tensor("ones", [1, P], f32r, kind="ExternalInput")
    yshard_d = nc.dram_tensor("yshard", [D * t_total // n_cores], f32,
                              kind="ExternalOutput")

    xT_r = xT_d.ap().rearrange("(do dp) t -> dp do t", dp=P)
    wgT_r = wgT_d.ap().rearrange("(do dp) i -> dp do i", dp=P)
    wuT_r = wuT_d.ap().rearrange("(do dp) i -> dp do i", dp=P)
    wdT_r = wdT_d.ap().rearrange("(io ip) d -> ip io d", ip=P)

    with tile.TileContext(nc) as tc_ctx:
        tcx = tc_ctx
        with tcx.tile_pool(name="wpool", bufs=1) as wpool, \
             tcx.tile_pool(name="xpool", bufs=2) as xpool, \
             tcx.tile_pool(name="hpool", bufs=2) as hpool, \
             tcx.tile_pool(name="ypool", bufs=2) as ypool, \
             tcx.tile_pool(name="gspool", bufs=3) as gspool, \
             tcx.tile_pool(name="gbpool", bufs=2) as gbpool, \
             tcx.tile_pool(name="psg", bufs=2, space="PSUM") as psg, \
             tcx.tile_pool(name="psu", bufs=2, space="PSUM") as psu, \
             tcx.tile_pool(name="psy", bufs=2, space="PSUM") as psy, \
             tcx.tile_pool(name="psb", bufs=1, space="PSUM") as psb, \
             tcx.tile_pool(name="dram", bufs=1, space="DRAM") as dram:

            # ---- resident weights ----
            wg_sb = wpool.tile([P, D_T, I], f32r)
            wu_sb = wpool.tile([P, D_T, I], f32r)
            wd_sb = wpool.tile([P, I_T, D], f32r)
            for d_o in range(D_T):
                nc.sync.dma_start(wg_sb[:, d_o, :], wgT_r[:, d_o, :])
                nc.sync.dma_start(wu_sb[:, d_o, :], wuT_r[:, d_o, :])
            for i_o in range(I_T):
                nc.sync.dma_start(wd_sb[:, i_o, :], wdT_r[:, i_o, :])

            # gating row + ones column for partition-broadcast matmul
            gcol_sb = wpool.tile([1, t_total], f32r)
            nc.sync.dma_start(gcol_sb[:], gcol_d.ap())
            ones_sb = wpool.tile([1, P], f32r)
            nc.sync.dma_start(ones_sb[:], ones_d.ap())

            partial = dram.tile([D, t_total], f32)
            partial_r = partial.rearrange("(do dp) t -> dp do t", dp=P)
            rs_out = dram.tile([D * t_total // n_cores], f32)

            for ci in range(n_chunks):
                t0 = ci * tc
                xt = xpool.tile([P, D_T, tc], f32r, tag="xt")
                half = D_T // 2
                nc.sync.dma_start(xt[:, :half, :], xT_r[:, :half, t0:t0 + tc])
                nc.sync.dma_start(xt[:, half:, :], xT_r[:, half:, t0:t0 + tc])

                # broadcast gating row to 128 partitions for this chunk
                gb_ps = psb.tile([P, tc], f32, tag="gbps")
                nc.tensor.matmul(gb_ps[:], ones_sb[:], gcol_sb[:, t0:t0 + tc],
                                 start=True, stop=True)
                gb_sb = gbpool.tile([P, tc], f32, tag="gb")
                nc.scalar.copy(out=gb_sb[:], in_=gb_ps[:])

                h = hpool.tile([P, I_T, tc], f32r, tag="h")
                for i_o in range(I_T):
                    pg = psg.tile([P, tc], f32, tag="pg")
                    pu = psu.tile([P, tc], f32, tag="pu")
                    for d_o in range(D_T):
                        nc.tensor.matmul(
                            pg[:], wg_sb[:, d_o, i_o * P:(i_o + 1) * P],
                            xt[:, d_o, :],
                            start=(d_o == 0), stop=(d_o == D_T - 1))
                    for d_o in range(D_T):
                        nc.tensor.matmul(
                            pu[:], wu_sb[:, d_o, i_o * P:(i_o + 1) * P],
                            xt[:, d_o, :],
                            start=(d_o == 0), stop=(d_o == D_T - 1))
                    gs = gspool.tile([P, tc], f32r, tag="gs")
                    nc.scalar.activation(gs[:], pg[:],
                                         mybir.ActivationFunctionType.Silu)
                    nc.vector.tensor_mul(out=h[:, i_o, :], in0=gs[:], in1=pu[:])

                yout = ypool.tile([P, D_T, tc], f32, tag="yout")
                for d_o in range(D_T):
                    py = psy.tile([P, tc], f32, tag="py")
                    for i_o in range(I_T):
                        nc.tensor.matmul(
                            py[:], wd_sb[:, i_o, d_o * P:(d_o + 1) * P],
                            h[:, i_o, :],
                            start=(i_o == 0), stop=(i_o == I_T - 1))
                    nc.vector.tensor_mul(out=yout[:, d_o, :], in0=py[:],
                                         in1=gb_sb[:])
                nc.sync.dma_start(partial_r[:, :half, t0:t0 + tc],
                                  yout[:, :half, :])
                nc.sync.dma_start(partial_r[:, half:, t0:t0 + tc],
                                  yout[:, half:, :])

            nc.gpsimd.collective_compute(
                "ReduceScatter", mybir.AluOpType.add,
                replica_groups=[list(range(n_cores))],
                ins=[partial[:].opt()], outs=[rs_out[:].opt()])

            shard = D * t_total // n_cores
            q = shard // 4
            for k in range(4):
                nc.sync.dma_start(yshard_d.ap()[k * q:(k + 1) * q],
                                  rs_out[k * q:(k + 1) * q])

    nc.compile()
    return nc


# ---------------------------------------------------------------------------
# Host-side wrapper

_CACHED = {}


def _get_nc(t_total=T, tc=TC, n_cores=N_CORES):
    key = (t_total, tc, n_cores)
    if key not in _CACHED:
        t0 = time.time()
        _CACHED[key] = build_nc(t_total, tc, n_cores)
        _log(f"built bass program in {time.time() - t0:.1f}s")
    return _CACHED[key]


def make_in_maps(x, gate_w, gate_proj_w, up_proj_w, down_proj_w,
                 t_total=T, n_cores=N_CORES):
    x2d = np.ascontiguousarray(np.asarray(x, dtype=np.float32).reshape(t_total, D))
    xT = round_f32r(x2d.T)  # [D, T]
    gating = host_gating(x2d, np.asarray(gate_w, dtype=np.float32))  # [T, E]
    gating_r = round_f32r(gating.T)  # [E, T]
    in_maps = []
    for c in range(n_cores):
        in_maps.append({
            "xT": xT,
            "wgT": round_f32r(np.asarray(gate_proj_w[c], np.float32).T),
            "wuT": round_f32r(np.asarray(up_proj_w[c], np.float32).T),
            "wdT": round_f32r(np.asarray(down_proj_w[c], np.float32).T),
            "gcol": gating_r[c:c + 1, :],
            "ones": np.ones((1, P), dtype=np.float32),
        })
    return in_maps


def assemble_output(results, t_total=T, n_cores=N_CORES):
    shard = D // n_cores
    yT = np.empty((D, t_total), dtype=np.float32)
    for c in range(n_cores):
        yT[c * shard:(c + 1) * shard, :] = \
            results[c]["yshard"].reshape(shard, t_total)
    return np.ascontiguousarray(yT.T).reshape(B, S, D)


def kernel(x, gate_w, gate_proj_w, up_proj_w, down_proj_w,
           num_experts_per_tok=2, _trace=False, _trace_cores=None):
    from concourse import bass_utils
    assert int(num_experts_per_tok) == TOPK
    nc = _get_nc()
    in_maps = make_in_maps(x, gate_w, gate_proj_w, up_proj_w, down_proj_w)
    kwargs = {}
    if _trace:
        try:
            sys.path.insert(0, os.path.dirname(os.path.abspath(__file__)))
            import axon_profile_shim
            axon_profile_shim.install()
        except Exception as exc:  # profiling is best-effort
            _log(f"profile shim unavailable: {exc}")
        kwargs = dict(trace=True,
                      trace_cores=_trace_cores or list(range(N_CORES)))
    t0 = time.time()
    res = bass_utils.run_bass_kernel_spmd(
        nc, in_maps, core_ids=list(range(N_CORES)), **kwargs)
    _log(f"run_bass_kernel_spmd took {time.time() - t0:.1f}s")
    kernel.last_result = res
    return assemble_output(res.results)


kernel.last_result = None
